# revision 1
# baseline (speedup 1.0000x reference)
"""Trainium2 Bass kernel for DPL safe-policy head.

Computes, for x:[B,H] and three tiny heads Wg/Wp/Wa (4/4/5 logits):
    ghost  = softmax(x@Wg + bg); pacman = softmax(x@Wp + bp); base = softmax(x@Wa + ba)
    unsafe[b,a] = sum_cd pacman[b,c] * T[a,c,d] * ghost[b,d]   (T fixed 0/1 tensor)
    out = base*(1-unsafe) / sum(...)

Closed form used on device (softmax normalizations cancel except ghost/pacman's,
which fold into Sp*Sg):
    E = exp(logits), Sg = sum(EG), Sp = sum(EP), SS = Sp*Sg
    u0 = sum_c EPc*EGc ; u1 = EP0*EG1+EP2*EG3 ; u2 = EP1*EG0+EP3*EG2
    t_j = EA_j * (SS - u_j)  (u3 = u4 = 0);  out_j = t_j / sum_j t_j

Sharding: pure data parallel over batch across 8 cores (2048 rows each).

Per core pipeline (memory-bound: stream x once from HBM; measured ~420GB/s
with full-tile 8KiB-line DMAs; PE LDWEIGHTS issue rate is the co-bottleneck,
so every engine has exactly one job):
  - x streams through the sync HWDGE queue as full-tile [128, 2048] DMAs
    (8 KiB lines); tiles 0 and 15 are split into quarters to shorten the
    startup ramp and the post-stream tail. The identity rides the gpsimd
    SWDGE queue up front; w (host-pretransposed so the load is one
    contiguous [128, 416B-line] transfer, not 2048 tiny descriptors) and
    the host-replicated bias follow it, landing before the first lagged
    accumulation matmuls need them. Anything queued on a HWDGE queue
    behind the x stream would crawl at ~20GB/s until ~14us.
  - ACT converts each tile to fp16 (the only full pass over x it does)
    plus the two tail exps
  - PE: per 128x128 chunk, one fp16 transpose (is_transpose matmul:
    LDWEIGHTS in transpose mode + identity stream, ~106ns cadence) and one
    fp16 accumulation matmul (N=13, ~27ns cadence); the bias is folded in
    by the DVE fold instead of a rank-1 matmul. Accum matmuls are emitted
    chunk-interleaved 16 chunks behind their transposes, and transposes
    get earlier scheduler priority (tc.high_priority), so the list
    scheduler keeps the PE covered with transpose work while DVE copies
    land - the PE now ends ~2us after the stream instead of ~9us.
  - DVE copies PSUM->SBUF fp16 transposed operands (group granularity,
    [128, 1024], one full PSUM bank) + per-tile bias-add fold (reads PSUM, which gpsimd cannot)
  - the logic-layer tail runs per half-batch: mid-kernel half on gpsimd
    (products) + DVE (reductions/reciprocal) so the copy stream barely
    pauses; the final, latency-exposed half entirely on DVE
  - output written as one contiguous [128, NT*5] block per half (160B
    partition lines; the old strided [.., t, j] store was 20B lines and
    cost a ~14us serial tail); host reorders to [B, 5]

fp16 single-term matmul (f16x1): max rel err ~1.5e-3 vs the fp32 reference
(test gate 2e-3, harness gate 2e-2).

History: 95.1us (f16x3 baseline) -> ~76us: dropped the 3-term matmul after
verifying f16x1 numerics in numpy (1.46e-3), pre-convert to fp16 so
transposes are 1-pass, contiguous output layout, engine re-assignment as
above, 8KiB DMA lines (355 -> 420GB/s), constants off the stream's queue.
Per-core ~72-77us; run-to-run spread is throttle noise (throttle_active
counters in the NTFF summary).
"""

import numpy as np

import concourse.bacc as bacc
import concourse.mybir as mybir
import concourse.tile as tile
from concourse.bass_utils import run_bass_kernel_spmd

F32 = mybir.dt.float32
F16 = mybir.dt.float16
AX = mybir.AxisListType
ADD = mybir.AluOpType.add

MODE = "f16pre"

N_CORES = 8
B_FULL, H = 16384, 2048
B = B_FULL // N_CORES  # rows per core
P = 128
NT = B // P            # batch tiles per core
NCH = H // P           # contraction chunks
GC = 8                 # chunks per psum transpose group
NG = NCH // GC
J = 13                 # 4 + 4 + 5 logits


def _build_program(mode):
    assert mode == "f16pre"
    nc = bacc.Bacc("TRN2", target_bir_lowering=False, debug=False,
                   num_devices=N_CORES)
    x_d = nc.dram_tensor("x", [B, H], F32, kind="ExternalInput")
    w_d = nc.dram_tensor("w", [P, NCH * J], F16, kind="ExternalInput")
    b_d = nc.dram_tensor("b", [P, J], F32, kind="ExternalInput")
    e_d = nc.dram_tensor("ident", [P, P], F16, kind="ExternalInput")
    y_d = nc.dram_tensor("y", [P, NT * 5], F32, kind="ExternalOutput")

    with tile.TileContext(nc) as tc:
        with (
            tc.tile_pool(name="const", bufs=1) as cpool,
            tc.tile_pool(name="xin", bufs=8) as xin_pool,
            tc.tile_pool(name="xinq", bufs=4) as xinq_pool,
            tc.tile_pool(name="hiq", bufs=4) as hiq_pool,
            tc.tile_pool(name="hi", bufs=8) as hi_pool,
            tc.tile_pool(name="xt", bufs=4) as xt_pool,
            tc.tile_pool(name="tp", bufs=6, space="PSUM") as tp_pool,
            tc.tile_pool(name="acc", bufs=2, space="PSUM") as acc_pool,
            tc.tile_pool(name="work", bufs=1) as wpool,
            tc.tile_pool(name="tailp", bufs=2) as tpool,
        ):
            # Constants ride the gpsimd SWDGE queue (identity now, w/b
            # emitted in the t-loop below): transfers queued on a HWDGE
            # queue behind the x stream crawl at ~20GB/s until ~14us, while
            # the SWDGE trickle interleaves and lands everything before the
            # first lagged accumulation matmuls and folds need it.
            id_sb = cpool.tile([P, P], F16)
            nc.gpsimd.dma_start(id_sb[:], e_d.ap())
            w_sb = cpool.tile([P, NCH, J], F16)
            b_sb = cpool.tile([P, J], F32)

            # logits staging, one tile per half so each half's tail only
            # depends on its own 8 batch tiles
            NTH = NT // 2
            all_st = [wpool.tile([P, NTH, J], F32, tag=f"st{h}",
                                 name=f"all_st{h}")
                      for h in range(2)]

            def tail(h):
                # The mid-kernel tail (h=0) splits elementwise products onto
                # the idle gpsimd engine so DVE's copy stream barely pauses
                # (free-dim reductions + reciprocal must stay on DVE). The
                # final tail (h=1) is latency-exposed, so it runs entirely
                # on DVE to avoid ~6 cross-engine semaphore hops, with the
                # output DMA on the (by then idle) scalar HWDGE queue.
                ew = nc.vector if h == 1 else nc.gpsimd
                st = all_st[h][:]
                e_all = tpool.tile([P, NTH, J], F32, tag="e_all")
                nc.scalar.activation(e_all[:], st,
                                     mybir.ActivationFunctionType.Exp)
                EG = e_all[:, :, 0:4]
                EP = e_all[:, :, 4:8]
                EA = e_all[:, :, 8:13]

                tmp4 = tpool.tile([P, NTH, 4], F32, tag="tmp4")
                ew.tensor_mul(tmp4[:], EP, EG)
                tmp2 = tpool.tile([P, NTH, 2], F32, tag="tmp2")
                ew.tensor_mul(tmp2[:], e_all[:, :, 4:8:2],
                              e_all[:, :, 1:4:2])
                tmp2b = tpool.tile([P, NTH, 2], F32, tag="tmp2b")
                ew.tensor_mul(tmp2b[:], e_all[:, :, 5:8:2],
                              e_all[:, :, 0:3:2])

                sg = tpool.tile([P, NTH], F32, tag="sg")
                nc.vector.tensor_reduce(sg[:], EG, axis=AX.X, op=ADD)
                sp = tpool.tile([P, NTH], F32, tag="sp")
                nc.vector.tensor_reduce(sp[:], EP, axis=AX.X, op=ADD)
                u3 = tpool.tile([P, NTH, 3], F32, tag="u3")
                nc.vector.tensor_reduce(u3[:, :, 0], tmp4[:], axis=AX.X,
                                        op=ADD)
                nc.vector.tensor_reduce(u3[:, :, 1], tmp2[:], axis=AX.X,
                                        op=ADD)
                nc.vector.tensor_reduce(u3[:, :, 2], tmp2b[:], axis=AX.X,
                                        op=ADD)

                ss = tpool.tile([P, NTH], F32, tag="ss")
                ew.tensor_mul(ss[:], sp[:], sg[:])
                V = tpool.tile([P, NTH, 5], F32, tag="V")
                ew.tensor_sub(V[:, :, 0:3],
                              ss[:].broadcast_to([P, NTH, 3]), u3[:])
                ew.tensor_copy(V[:, :, 3:5],
                               ss[:].broadcast_to([P, NTH, 2]))
                tj = tpool.tile([P, NTH, 5], F32, tag="tj")
                ew.tensor_mul(tj[:], EA, V[:])

                s5 = tpool.tile([P, NTH], F32, tag="s5")
                nc.vector.tensor_reduce(s5[:], tj[:], axis=AX.X, op=ADD)
                r5 = tpool.tile([P, NTH], F32, tag="r5")
                nc.vector.reciprocal(r5[:], s5[:])

                out_sb = tpool.tile([P, NTH, 5], F32, tag="out_sb")
                ew.tensor_mul(out_sb[:], tj[:],
                              r5[:].broadcast_to([P, NTH, 5]))
                # contiguous [128, 40] block (160B lines, 320B stride)
                out_eng = nc.scalar if h == 1 else nc.gpsimd
                out_eng.dma_start(
                    y_d.ap()[:, h * NTH * 5:(h + 1) * NTH * 5],
                    out_sb[:].rearrange("p t j -> p (t j)"))

            def fold(t, acc):
                # PSUM->SBUF logits stage with the bias add fused, on DVE
                # (gpsimd cannot read PSUM; on ACT it would stall the
                # in-order convert stream whenever the PE lags)
                nc.vector.tensor_add(all_st[t // NTH][:, t % NTH, :],
                                     acc[:], b_sb[:])
                if t % NTH == NTH - 1:
                    tail(t // NTH)

            def emit_one(item):
                t, c, acc, xt, k = item
                nc.tensor.matmul(acc[:], xt[:, k * P:(k + 1) * P],
                                 w_sb[:, c, :],
                                 start=c == 0, stop=c == NCH - 1,
                                 skip_group_check=True)
                if c == NCH - 1:
                    fold(t, acc)

            # Chunk-level software pipelining: each accumulation matmul is
            # emitted interleaved between transposes, two groups (16
            # chunks) after its own transpose, so its DVE copy (PSUM read +
            # semaphore hops, ~750ns latency) is long complete by the time
            # the in-order PE stream reaches it - even after the tile
            # scheduler reclumps the stream into 8T+8M runs.
            pend = []
            LAGC = 16

            for t in range(NT):
                # full-tile transfers (8 KiB lines) for HBM burst efficiency;
                # tiles 0 and NT-1 use quarter tiles: tile 0 so the first
                # convert/transpose chain starts ~1.5us earlier during the
                # DMA ramp, the last tile so the tail chain isn't gated on a
                # full-tile (1.7us) convert after the stream ends.
                NP = 4 if t in (0, NT - 1) else 1
                CW_ = H // NP
                hq = []
                for q in range(NP):
                    xp = xinq_pool if NP == 4 else xin_pool
                    xqt = xp.tile([P, CW_], F32, tag=f"xh{NP}_{q}",
                                  name=f"xh{t}_{q}")
                    nc.sync.dma_start(
                        xqt[:],
                        x_d.ap()[t * P:(t + 1) * P,
                                 q * CW_:(q + 1) * CW_])
                    if t == 0 and q == 1:
                        # w/b on the SWDGE queue behind the identity; their
                        # slow trickle still lands before the first lagged
                        # accumulation matmuls and folds need them
                        nc.gpsimd.dma_start(
                            w_sb[:].rearrange("p c j -> p (c j)"), w_d.ap())
                        nc.gpsimd.dma_start(b_sb[:], b_d.ap())
                    hp = hiq_pool if NP == 4 else hi_pool
                    hqt = hp.tile([P, CW_], F16, tag=f"hh{NP}_{q}",
                                  name=f"hh{t}_{q}")
                    nc.scalar.copy(hqt[:], xqt[:])  # fp16 round on ACT
                    hq.append(hqt)

                def chunk(c, hq=hq, cpp=CW_ // P):
                    return hq[c // cpp][:, (c % cpp) * P:(c % cpp + 1) * P]

                acc = acc_pool.tile([P, J], F32)
                for g in range(NG):
                    tp = tp_pool.tile([P, GC * P], F16)
                    for k in range(GC):
                        c = GC * g + k
                        # transposes get earlier scheduler priority so the
                        # list scheduler (whose cost model thinks copies are
                        # fast) keeps PE busy with transposes instead of
                        # parking accum matmuls right behind a fresh copy
                        with tc.high_priority(offset=100):
                            nc.tensor.transpose(
                                tp[:, k * P:(k + 1) * P],
                                chunk(c),
                                id_sb[:])
                        if len(pend) > LAGC:
                            emit_one(pend.pop(0))
                    xt = xt_pool.tile([P, GC * P], F16, tag="xt")
                    nc.vector.tensor_copy(xt[:], tp[:])  # PSUM->SBUF
                    for k in range(GC):
                        pend.append((t, GC * g + k, acc, xt, k))
            for p in pend:
                emit_one(p)

    nc.compile()
    return nc


_NC_CACHE = {}


def _get_program(mode=MODE):
    if mode not in _NC_CACHE:
        _NC_CACHE[mode] = _build_program(mode)
    return _NC_CACHE[mode]


def _prep_in_maps(x, Wg, bg, Wp, bp, Wa, ba, mode=MODE):
    x = np.ascontiguousarray(np.asarray(x, dtype=np.float32))
    W = np.concatenate([np.asarray(Wg), np.asarray(Wp), np.asarray(Wa)],
                       axis=1).astype(np.float32)
    bvec = np.concatenate([np.asarray(bg), np.asarray(bp), np.asarray(ba)]
                          ).astype(np.float32).reshape(1, J)
    ident = np.eye(P, dtype=np.float16)
    # [h, j] -> [p, c*J+j] with h = c*128 + p (contiguous device load)
    w_dev = np.ascontiguousarray(
        W.astype(np.float16).reshape(NCH, P, J).transpose(1, 0, 2)
    ).reshape(P, NCH * J)
    b_dev = np.ascontiguousarray(np.broadcast_to(bvec, (P, J)),
                                 dtype=np.float32)
    in_maps = []
    for i in range(N_CORES):
        in_maps.append({
            "x": x[i * B:(i + 1) * B],
            "w": w_dev,
            "b": b_dev,
            "ident": ident,
        })
    return in_maps


def kernel(x, Wg, bg, Wp, bp, Wa, ba):
    in_maps = _prep_in_maps(x, Wg, bg, Wp, bp, Wa, ba)
    nc = _get_program()
    res = run_bass_kernel_spmd(nc, in_maps, core_ids=list(range(N_CORES)))
    outs = []
    for i in range(N_CORES):
        y = res.results[i]["y"]  # [P, NT*5], row b = t*P + p at [p, t*5+j]
        outs.append(y.reshape(P, NT, 5).transpose(1, 0, 2).reshape(B, 5))
    return np.concatenate(outs, axis=0)



# revision 2
# speedup vs baseline: 1.0109x; 1.0109x over previous
"""Trainium2 Bass kernel for DPL safe-policy head.

Computes, for x:[B,H] and three tiny heads Wg/Wp/Wa (4/4/5 logits):
    ghost  = softmax(x@Wg + bg); pacman = softmax(x@Wp + bp); base = softmax(x@Wa + ba)
    unsafe[b,a] = sum_cd pacman[b,c] * T[a,c,d] * ghost[b,d]   (T fixed 0/1 tensor)
    out = base*(1-unsafe) / sum(...)

Closed form used on device (softmax normalizations cancel except ghost/pacman's,
which fold into Sp*Sg):
    E = exp(logits), Sg = sum(EG), Sp = sum(EP), SS = Sp*Sg
    u0 = sum_c EPc*EGc ; u1 = EP0*EG1+EP2*EG3 ; u2 = EP1*EG0+EP3*EG2
    t_j = EA_j * (SS - u_j)  (u3 = u4 = 0);  out_j = t_j / sum_j t_j

Sharding: pure data parallel over batch across 8 cores (2048 rows each).

Per core pipeline (memory-bound; the x stream at ~420GB/s on the sync HWDGE
ring is the roofline, every other engine is kept strictly under it):
  - x streams through the sync HWDGE queue as full-tile [128, 2048] DMAs
    (8 KiB lines); tiles 0 and 15 are split into quarters to shorten the
    startup ramp and the post-stream tail.
  - constants ride the scalar HWDGE ring (NOT gpsimd SWDGE: SWDGE's
    descriptor rings sit on partition 0-31 AXI ports and contend with SDMA
    engines 7/15 - measured as DMA engine 15 finishing its share of the x
    stream ~8us after the others). identity is the ACT engine's first
    instruction so it lands before the first transpose; w/b issue after
    tile 0's converts, still ~2 tiles before the first lagged accum needs w.
  - ACT converts each tile to fp16 (2.0us per full tile; ACT is the #2
    engine at ~37us busy and must never be starved or delayed - DVE/gpsimd
    CASTs measured 4x slower, so no offload) plus the per-group tail exps.
  - PE: per 128x128 chunk, one fp16 transpose (~58ns solo cadence) and one
    fp16 accumulation matmul (~35ns solo; LDWEIGHTS gets FWL). Accum
    matmuls are emitted chunk-interleaved 16 chunks behind their
    transposes with transposes at higher scheduler priority; composed
    cadence measured 103ns/chunk - PE ~27us busy, under the stream.
  - DVE copies PSUM->SBUF fp16 transposed operands ([128, 1024] groups,
    0.67ns/elem, ~22us total) + per-tile bias-add fold (reads PSUM).
  - the logic-layer tail runs per quarter (4 tiles): groups 0-2 put the
    elementwise products on gpsimd (reductions/reciprocal stay on DVE),
    the final latency-exposed group runs entirely on DVE.
  - outputs: one contiguous [128, 20] block per quarter, all emitted after
    the main loop on the sync ring (they drain after the x backlog, which
    is fine - nothing downstream reads them; the last one issues when the
    ring is empty).

fp16 single-term matmul (f16x1): max rel err ~1.5e-3 vs the fp32 reference
(test gate 2e-3, harness gate 2e-2).

History: 95.1us (f16x3) -> ~77us (f16x1, engine reassignment, 8KiB lines)
-> this version: constants off SWDGE, 4 quarter-tails, outputs last on the
sync ring.
"""

import numpy as np

import concourse.bacc as bacc
import concourse.mybir as mybir
import concourse.tile as tile
from concourse.bass_utils import run_bass_kernel_spmd

F32 = mybir.dt.float32
F16 = mybir.dt.float16
AX = mybir.AxisListType
ADD = mybir.AluOpType.add

MODE = "f16pre"

N_CORES = 8
B_FULL, H = 16384, 2048
B = B_FULL // N_CORES  # rows per core
P = 128
NT = B // P            # batch tiles per core
NCH = H // P           # contraction chunks
GC = 8                 # chunks per psum transpose group
NG = NCH // GC
J = 13                 # 4 + 4 + 5 logits
NTAILS = 4             # tail groups
NTQ = NT // NTAILS     # tiles per tail group


def _build_program(mode):
    assert mode == "f16pre"
    nc = bacc.Bacc("TRN2", target_bir_lowering=False, debug=False,
                   num_devices=N_CORES)
    x_d = nc.dram_tensor("x", [B, H], F32, kind="ExternalInput")
    w_d = nc.dram_tensor("w", [P, NCH * J], F16, kind="ExternalInput")
    b_d = nc.dram_tensor("b", [P, J], F32, kind="ExternalInput")
    e_d = nc.dram_tensor("ident", [P, P], F16, kind="ExternalInput")
    y_d = nc.dram_tensor("y", [P, NT * 5], F32, kind="ExternalOutput")

    with tile.TileContext(nc) as tc:
        with (
            tc.tile_pool(name="const", bufs=1) as cpool,
            tc.tile_pool(name="xin", bufs=8) as xin_pool,
            tc.tile_pool(name="xinq", bufs=4) as xinq_pool,
            tc.tile_pool(name="hiq", bufs=4) as hiq_pool,
            tc.tile_pool(name="hi", bufs=8) as hi_pool,
            tc.tile_pool(name="xt", bufs=4) as xt_pool,
            tc.tile_pool(name="tp", bufs=6, space="PSUM") as tp_pool,
            tc.tile_pool(name="acc", bufs=2, space="PSUM") as acc_pool,
            tc.tile_pool(name="work", bufs=1) as wpool,
            tc.tile_pool(name="tailp", bufs=2) as tpool,
        ):
            # identity is the ACT engine's first instruction: issued ~7.2us,
            # lands ~8.5us, first transpose needs it ~10us.
            id_sb = cpool.tile([P, P], F16)
            nc.scalar.dma_start(id_sb[:], e_d.ap())
            w_sb = cpool.tile([P, NCH, J], F16)
            b_sb = cpool.tile([P, J], F32)

            # per-group logits staging
            all_st = [wpool.tile([P, NTQ, J], F32, tag=f"st{g}",
                                 name=f"all_st{g}")
                      for g in range(NTAILS)]
            out_tiles = [wpool.tile([P, NTQ, 5], F32, tag=f"ot{g}",
                                    name=f"out_sb{g}")
                         for g in range(NTAILS)]
            out_dmas = []

            def tail(g):
                # groups 0-2: elementwise products on the idle gpsimd so
                # DVE's copy stream barely pauses (free-dim reductions +
                # reciprocal must stay on DVE). The final group is
                # latency-exposed: all-DVE avoids cross-engine hops.
                ew = nc.vector if g == NTAILS - 1 else nc.gpsimd
                st = all_st[g][:]
                e_all = tpool.tile([P, NTQ, J], F32, tag="e_all")
                nc.scalar.activation(e_all[:], st,
                                     mybir.ActivationFunctionType.Exp)
                EG = e_all[:, :, 0:4]
                EP = e_all[:, :, 4:8]
                EA = e_all[:, :, 8:13]

                tmp4 = tpool.tile([P, NTQ, 4], F32, tag="tmp4")
                ew.tensor_mul(tmp4[:], EP, EG)
                tmp2 = tpool.tile([P, NTQ, 2], F32, tag="tmp2")
                ew.tensor_mul(tmp2[:], e_all[:, :, 4:8:2],
                              e_all[:, :, 1:4:2])
                tmp2b = tpool.tile([P, NTQ, 2], F32, tag="tmp2b")
                ew.tensor_mul(tmp2b[:], e_all[:, :, 5:8:2],
                              e_all[:, :, 0:3:2])

                sg = tpool.tile([P, NTQ], F32, tag="sg")
                nc.vector.tensor_reduce(sg[:], EG, axis=AX.X, op=ADD)
                sp = tpool.tile([P, NTQ], F32, tag="sp")
                nc.vector.tensor_reduce(sp[:], EP, axis=AX.X, op=ADD)
                u3 = tpool.tile([P, NTQ, 3], F32, tag="u3")
                nc.vector.tensor_reduce(u3[:, :, 0], tmp4[:], axis=AX.X,
                                        op=ADD)
                nc.vector.tensor_reduce(u3[:, :, 1], tmp2[:], axis=AX.X,
                                        op=ADD)
                nc.vector.tensor_reduce(u3[:, :, 2], tmp2b[:], axis=AX.X,
                                        op=ADD)

                ss = tpool.tile([P, NTQ], F32, tag="ss")
                ew.tensor_mul(ss[:], sp[:], sg[:])
                V = tpool.tile([P, NTQ, 5], F32, tag="V")
                ew.tensor_sub(V[:, :, 0:3],
                              ss[:].broadcast_to([P, NTQ, 3]), u3[:])
                ew.tensor_copy(V[:, :, 3:5],
                               ss[:].broadcast_to([P, NTQ, 2]))
                tj = tpool.tile([P, NTQ, 5], F32, tag="tj")
                ew.tensor_mul(tj[:], EA, V[:])

                s5 = tpool.tile([P, NTQ], F32, tag="s5")
                nc.vector.tensor_reduce(s5[:], tj[:], axis=AX.X, op=ADD)
                r5 = tpool.tile([P, NTQ], F32, tag="r5")
                nc.vector.reciprocal(r5[:], s5[:])

                out_sb = out_tiles[g]
                ew.tensor_mul(out_sb[:], tj[:],
                              r5[:].broadcast_to([P, NTQ, 5]))
                # contiguous [128, 20] block per group; DMA emitted after
                # the main loop so the sync engine's x issues come first
                out_dmas.append((g, out_sb))

            def fold(t, acc):
                # PSUM->SBUF logits stage with the bias add fused, on DVE
                # (gpsimd cannot read PSUM; ACT must not stall)
                nc.vector.tensor_add(all_st[t // NTQ][:, t % NTQ, :],
                                     acc[:], b_sb[:])
                if t % NTQ == NTQ - 1:
                    tail(t // NTQ)

            def emit_one(item):
                t, c, acc, xt, k = item
                nc.tensor.matmul(acc[:], xt[:, k * P:(k + 1) * P],
                                 w_sb[:, c, :],
                                 start=c == 0, stop=c == NCH - 1,
                                 skip_group_check=True)
                if c == NCH - 1:
                    fold(t, acc)

            # Chunk-level software pipelining: each accumulation matmul is
            # emitted interleaved between transposes, two groups (16
            # chunks) after its own transpose, so its DVE copy (PSUM read +
            # semaphore hops, ~750ns latency) is long complete by the time
            # the in-order PE stream reaches it.
            pend = []
            LAGC = 16

            for t in range(NT):
                # full-tile transfers (8 KiB lines) for HBM burst
                # efficiency; tiles 0 and NT-1 use quarter tiles so the
                # first convert starts during the DMA ramp and the tail
                # chain isn't gated on a full-tile (2us) convert.
                NP = 4 if t in (0, NT - 1) else 1
                CW_ = H // NP
                hq = []
                for q in range(NP):
                    xp = xinq_pool if NP == 4 else xin_pool
                    xqt = xp.tile([P, CW_], F32, tag=f"xh{NP}_{q}",
                                  name=f"xh{t}_{q}")
                    nc.sync.dma_start(
                        xqt[:],
                        x_d.ap()[t * P:(t + 1) * P,
                                 q * CW_:(q + 1) * CW_])
                    hp = hiq_pool if NP == 4 else hi_pool
                    hqt = hp.tile([P, CW_], F16, tag=f"hh{NP}_{q}",
                                  name=f"hh{t}_{q}")
                    nc.scalar.copy(hqt[:], xqt[:])  # fp16 round on ACT
                    hq.append(hqt)
                if t == 0:
                    # w/b on the scalar ring right after tile 0's converts:
                    # issued ~12us, lands well before the first lagged
                    # accum matmul (~13us) and the first fold need them.
                    nc.scalar.dma_start(
                        w_sb[:].rearrange("p c j -> p (c j)"), w_d.ap())
                    nc.scalar.dma_start(b_sb[:], b_d.ap())

                def chunk(c, hq=hq, cpp=CW_ // P):
                    return hq[c // cpp][:, (c % cpp) * P:(c % cpp + 1) * P]

                acc = acc_pool.tile([P, J], F32)
                for g in range(NG):
                    tp = tp_pool.tile([P, GC * P], F16)
                    for k in range(GC):
                        c = GC * g + k
                        # transposes get earlier scheduler priority so the
                        # list scheduler keeps PE busy with transposes
                        # instead of parking accum matmuls behind a fresh
                        # DVE copy
                        with tc.high_priority(offset=100):
                            nc.tensor.transpose(
                                tp[:, k * P:(k + 1) * P],
                                chunk(c),
                                id_sb[:])
                        if len(pend) > LAGC:
                            emit_one(pend.pop(0))
                    xt = xt_pool.tile([P, GC * P], F16, tag="xt")
                    nc.vector.tensor_copy(xt[:], tp[:])  # PSUM->SBUF
                    for k in range(GC):
                        pend.append((t, GC * g + k, acc, xt, k))
            for p in pend:
                emit_one(p)

            # output DMAs last: the sync engine has finished its x issues,
            # and the ring drains them after the x backlog.
            for g, out_sb in out_dmas:
                nc.sync.dma_start(
                    y_d.ap()[:, g * NTQ * 5:(g + 1) * NTQ * 5],
                    out_sb[:].rearrange("p t j -> p (t j)"))

    nc.compile()
    return nc


_NC_CACHE = {}


def _get_program(mode=MODE):
    if mode not in _NC_CACHE:
        _NC_CACHE[mode] = _build_program(mode)
    return _NC_CACHE[mode]


def _prep_in_maps(x, Wg, bg, Wp, bp, Wa, ba, mode=MODE):
    x = np.ascontiguousarray(np.asarray(x, dtype=np.float32))
    W = np.concatenate([np.asarray(Wg), np.asarray(Wp), np.asarray(Wa)],
                       axis=1).astype(np.float32)
    bvec = np.concatenate([np.asarray(bg), np.asarray(bp), np.asarray(ba)]
                          ).astype(np.float32).reshape(1, J)
    ident = np.eye(P, dtype=np.float16)
    # [h, j] -> [p, c*J+j] with h = c*128 + p (contiguous device load)
    w_dev = np.ascontiguousarray(
        W.astype(np.float16).reshape(NCH, P, J).transpose(1, 0, 2)
    ).reshape(P, NCH * J)
    b_dev = np.ascontiguousarray(np.broadcast_to(bvec, (P, J)),
                                 dtype=np.float32)
    in_maps = []
    for i in range(N_CORES):
        in_maps.append({
            "x": x[i * B:(i + 1) * B],
            "w": w_dev,
            "b": b_dev,
            "ident": ident,
        })
    return in_maps


def kernel(x, Wg, bg, Wp, bp, Wa, ba):
    in_maps = _prep_in_maps(x, Wg, bg, Wp, bp, Wa, ba)
    nc = _get_program()
    res = run_bass_kernel_spmd(nc, in_maps, core_ids=list(range(N_CORES)))
    outs = []
    for i in range(N_CORES):
        y = res.results[i]["y"]  # [P, NT*5], row b = t*P + p at [p, t*5+j]
        outs.append(y.reshape(P, NT, 5).transpose(1, 0, 2).reshape(B, 5))
    return np.concatenate(outs, axis=0)
